# revision 2
# baseline (speedup 1.0000x reference)
"""MoE gate kernel for Trainium2 (Bass/Tile), 8-core data-parallel.

Computes, for x:[4,4096,2048], W:[64,2048], b:[64]:
    logits = x @ W.T + b            -> [B,S,64]
    top2 vals/idx over experts
    gate   = softmax(scatter(top2 vals, -inf elsewhere))  (zeros off top-2)
Returns (gate:[4,4096,64] f32, top_idx:[4,4096,2] i32).

Sharding: tokens (B*S=16384) split 8 ways -> 2048 tokens/core; W/b replicated.
Host pre-transposes each core's x shard to d-major tiles so device DMAs are
fully contiguous 1MiB streams.
"""

import os
import sys

import numpy as np

for _p in ("/opt/trn_rl_repo",):
    if _p not in sys.path and os.path.isdir(_p):
        sys.path.insert(0, _p)

import concourse.bacc as bacc
import concourse.mybir as mybir
from concourse.bass_utils import run_bass_kernel_spmd
from concourse.tile import TileContext

B, S, DM, E, TOPK = 4, 4096, 2048, 64, 2
NCORES = 8
TOK = B * S            # 16384 tokens total
TPC = TOK // NCORES    # 2048 tokens per core
P = 128                # partitions
NT = TPC // P          # 16 token tiles per core
ND = DM // P           # 16 contraction chunks

_CACHE = {}

# exec time (ns) of the last traced run, for test harnesses
last_exec_time_ns = None


def _build():
    f32 = mybir.dt.float32
    nc = bacc.Bacc(None, target_bir_lowering=False)
    xin = nc.dram_tensor("xin", [NT, P, DM], f32, kind="ExternalInput")
    win = nc.dram_tensor("win", [P, ND * E], f32, kind="ExternalInput")
    bin_ = nc.dram_tensor("bin", [1, E], f32, kind="ExternalInput")
    gate = nc.dram_tensor("gate", [TPC, E], f32, kind="ExternalOutput")
    idx = nc.dram_tensor("idx", [TPC, TOPK], mybir.dt.int32, kind="ExternalOutput")

    with TileContext(nc) as tc:
        with tc.tile_pool(name="const", bufs=1) as cpool, \
             tc.tile_pool(name="xp", bufs=4) as xpool, \
             tc.tile_pool(name="ps", bufs=4, space="PSUM") as ppool, \
             tc.tile_pool(name="wk", bufs=3) as wpool:
            # Replicated gate weight, laid out [p, (k e)] so chunk k of the
            # contraction reads wsb[:, k*E:(k+1)*E] with partition = d-in-chunk.
            wsb = cpool.tile([P, ND * E], f32)
            nc.sync.dma_start(out=wsb[:], in_=win[:])
            bsb = cpool.tile([1, E], f32)
            nc.sync.dma_start(out=bsb[:], in_=bin_[:])
            ones = cpool.tile([1, P], f32)
            nc.vector.memset(ones[:], 1.0)

            for t in range(NT):
                # x tile for 128 tokens: [p=d-in-chunk, (k tokens)] — 1MiB
                # contiguous HBM read (host pre-arranged).
                xsb = xpool.tile([P, DM], f32)
                nc.sync.dma_start(out=xsb[:], in_=xin[t])

                pt = ppool.tile([P, E], f32, tag="psum")
                for k in range(ND):
                    nc.tensor.matmul(
                        pt[:],
                        lhsT=xsb[:, k * P:(k + 1) * P],
                        rhs=wsb[:, k * E:(k + 1) * E],
                        start=(k == 0),
                        stop=False,
                    )
                # bias via rank-1 matmul: ones[1,128]^T @ b[1,64]
                nc.tensor.matmul(pt[:], lhsT=ones[:, :], rhs=bsb[:, :],
                                 start=False, stop=True)

                lg = wpool.tile([P, E], f32)
                nc.scalar.activation(lg[:], pt[:],
                                     mybir.ActivationFunctionType.Copy)

                mx = wpool.tile([P, 8], f32)
                mi = wpool.tile([P, 8], mybir.dt.uint32)
                nc.vector.max(out=mx[:], in_=lg[:])
                nc.vector.max_index(out=mi[:], in_max=mx[:], in_values=lg[:])

                ix = wpool.tile([P, TOPK], mybir.dt.int32)
                nc.vector.tensor_copy(out=ix[:], in_=mi[:, 0:TOPK])
                nc.sync.dma_start(out=idx[t * P:(t + 1) * P, :], in_=ix[:])

                # softmax over the two kept logits: p1 = 1/(1+e), p2 = e/(1+e)
                # with e = exp(v2 - v1)
                sm = wpool.tile([P, 5], f32)
                d21 = sm[:, 0:1]
                e2 = sm[:, 1:2]
                den = sm[:, 2:3]
                rr = sm[:, 3:4]
                p2 = sm[:, 4:5]
                nc.vector.tensor_sub(d21, mx[:, 1:2], mx[:, 0:1])
                nc.scalar.activation(e2, d21, mybir.ActivationFunctionType.Exp)
                nc.vector.tensor_scalar_add(den, e2, 1.0)
                nc.vector.reciprocal(rr, den)
                nc.vector.tensor_mul(p2, e2, rr)

                # gate row = (lg==v1)*p1 + (lg==v2)*p2  (zeros elsewhere)
                g1 = wpool.tile([P, E], f32)
                g2 = wpool.tile([P, E], f32)
                nc.vector.scalar_tensor_tensor(
                    out=g1[:], in0=lg[:], scalar=mx[:, 0:1],
                    in1=rr.to_broadcast([P, E]),
                    op0=mybir.AluOpType.is_equal, op1=mybir.AluOpType.mult)
                nc.vector.scalar_tensor_tensor(
                    out=g2[:], in0=lg[:], scalar=mx[:, 1:2],
                    in1=p2.to_broadcast([P, E]),
                    op0=mybir.AluOpType.is_equal, op1=mybir.AluOpType.mult)
                nc.vector.tensor_add(g1[:], g1[:], g2[:])
                nc.sync.dma_start(out=gate[t * P:(t + 1) * P, :], in_=g1[:])
    if not nc.is_finalized():
        nc.finalize()
    return nc


def kernel(x, W, b):
    global last_exec_time_ns
    nc = _CACHE.get("nc")
    if nc is None:
        nc = _build()
        _CACHE["nc"] = nc

    xf = np.ascontiguousarray(np.asarray(x, dtype=np.float32)).reshape(TOK, DM)
    Wf = np.asarray(W, dtype=np.float32)
    bf = np.asarray(b, dtype=np.float32)

    # win[p, k*E+e] = W[e, k*P+p]
    warr = np.ascontiguousarray(
        Wf.T.reshape(ND, P, E).transpose(1, 0, 2)).reshape(P, ND * E)
    barr = np.ascontiguousarray(bf.reshape(1, E))

    in_maps = []
    for c in range(NCORES):
        xc = xf[c * TPC:(c + 1) * TPC]                 # [2048, 2048]
        # xin[t, p, k*P+j] = x[t*P+j, k*P+p]
        xt = np.ascontiguousarray(
            xc.reshape(NT, P, ND, P).transpose(0, 3, 2, 1)).reshape(NT, P, DM)
        in_maps.append({"xin": xt, "win": warr, "bin": barr})

    trace = bool(int(os.environ.get("KERNEL_TRACE", "0")))
    res = run_bass_kernel_spmd(nc, in_maps, list(range(NCORES)), trace=trace)
    last_exec_time_ns = res.exec_time_ns

    gate_full = np.concatenate(
        [res.results[c]["gate"] for c in range(NCORES)], axis=0)
    idx_full = np.concatenate(
        [res.results[c]["idx"] for c in range(NCORES)], axis=0)
    return (gate_full.reshape(B, S, E),
            idx_full.reshape(B, S, TOPK).astype(np.int32))


# revision 5
# speedup vs baseline: 1.4098x; 1.4098x over previous
"""MoE gate kernel for Trainium2 (Bass/Tile), 8-core data-parallel.

Computes, for x:[4,4096,2048], W:[64,2048], b:[64]:
    logits = x @ W.T + b            -> [B,S,64]
    top2 vals/idx over experts
    gate   = softmax(scatter(top2 vals, -inf elsewhere))  (zeros off top-2)
Returns (gate:[4,4096,64] f32, top_idx:[4,4096,2] i32).

Sharding: tokens (B*S=16384) split 8 ways -> 2048 tokens/core; W/b replicated.

Device strategy (form 2): keep the tiny gate weight stationary in the PE
array (64-column fp32 LDWEIGHTS) and stream x as the moving operand at
N=512, accumulating expert-major logits [64, 512] per token group in PSUM.
Then PE-transpose 128-token slices back to token-major [128, 64] for the
free-dim top-2 (max8/find_index8) and the masked sparse-softmax writes.
Host pre-permutes each core's x shard so every device DMA is a fully
contiguous 1MiB stream (d-chunk major).
"""

import os
import sys

import numpy as np

for _p in ("/opt/trn_rl_repo",):
    if _p not in sys.path and os.path.isdir(_p):
        sys.path.insert(0, _p)

import concourse.bacc as bacc
import concourse.mybir as mybir
from concourse.bass_utils import run_bass_kernel_spmd
from concourse.masks import make_identity
from concourse.tile import TileContext

B, S, DM, E, TOPK = 4, 4096, 2048, 64, 2
NCORES = 8
TOK = B * S            # 16384 tokens total
TPC = TOK // NCORES    # 2048 tokens per core
P = 128                # partitions
ND = DM // P           # 16 contraction chunks
NG = 4                 # token groups per core (512 tokens each)
GT = TPC // NG         # 512 tokens per group
NT = TPC // P          # 16 token tiles per core

_CACHE = {}

# exec time (ns) of the last traced run, for test harnesses
last_exec_time_ns = None


def _build():
    f32 = mybir.dt.float32
    nc = bacc.Bacc(None, target_bir_lowering=False)
    xin = nc.dram_tensor("xin", [ND, P, TPC], f32, kind="ExternalInput")
    win = nc.dram_tensor("win", [P, ND * E], f32, kind="ExternalInput")
    bin_ = nc.dram_tensor("bin", [1, E], f32, kind="ExternalInput")
    gate = nc.dram_tensor("gate", [TPC, E], f32, kind="ExternalOutput")
    idx = nc.dram_tensor("idx", [TPC, TOPK], mybir.dt.int32, kind="ExternalOutput")

    with TileContext(nc) as tc:
        with tc.tile_pool(name="const", bufs=1) as cpool, \
             tc.tile_pool(name="xp", bufs=4) as xpool, \
             tc.tile_pool(name="acc", bufs=1, space="PSUM") as apool, \
             tc.tile_pool(name="tp", bufs=3, space="PSUM") as tpool, \
             tc.tile_pool(name="wk", bufs=3) as wpool:
            # W^T chunks: wsb[:, k*E:(k+1)*E] = [d-in-chunk, expert]
            wsb = cpool.tile([P, ND * E], f32)
            nc.sync.dma_start(out=wsb[:], in_=win[:])
            bsb = cpool.tile([1, E], f32)
            nc.sync.dma_start(out=bsb[:], in_=bin_[:])
            ones = cpool.tile([1, GT], f32)
            nc.vector.memset(ones[:], 1.0)
            ident = cpool.tile([E, E], f32)
            make_identity(nc, ident[:])

            # expert-major logits accumulators, one per 512-token group
            pgs = [apool.tile([E, GT], f32, tag=f"pg{g}", name=f"pg{g}")
                   for g in range(NG)]

            for k in range(ND):
                xk = xpool.tile([P, TPC], f32)
                nc.sync.dma_start(out=xk[:], in_=xin[k])
                for g in range(NG):
                    nc.tensor.matmul(
                        pgs[g][:],
                        lhsT=wsb[:, k * E:(k + 1) * E],
                        rhs=xk[:, g * GT:(g + 1) * GT],
                        start=(k == 0),
                        stop=False,
                    )
            for g in range(NG):
                # bias via rank-1 matmul: b[1,64]^T @ ones[1,512]
                nc.tensor.matmul(pgs[g][:], lhsT=bsb[:, :], rhs=ones[:, :],
                                 start=False, stop=True)

            for g in range(NG):
                ls = wpool.tile([E, GT], f32, tag="ls")
                nc.scalar.activation(ls[:], pgs[g][:],
                                     mybir.ActivationFunctionType.Copy)
                for j in range(NG):
                    t = g * NG + j
                    ptt = tpool.tile([P, E], f32, tag="ptt")
                    nc.tensor.transpose(ptt[:], ls[:, j * P:(j + 1) * P],
                                        ident[:])

                    lg = wpool.tile([P, E], f32)
                    nc.scalar.activation(lg[:], ptt[:],
                                         mybir.ActivationFunctionType.Copy)

                    mx = wpool.tile([P, 8], f32)
                    mi = wpool.tile([P, 8], mybir.dt.uint32)
                    nc.vector.max(out=mx[:], in_=lg[:])
                    nc.vector.max_index(out=mi[:], in_max=mx[:], in_values=lg[:])

                    ix = wpool.tile([P, TOPK], mybir.dt.int32)
                    nc.vector.tensor_copy(out=ix[:], in_=mi[:, 0:TOPK])
                    nc.sync.dma_start(out=idx[t * P:(t + 1) * P, :], in_=ix[:])

                    # softmax over the two kept logits:
                    # p1 = 1/(1+e), p2 = e/(1+e), e = exp(v2 - v1)
                    sm = wpool.tile([P, 5], f32)
                    d21 = sm[:, 0:1]
                    e2 = sm[:, 1:2]
                    den = sm[:, 2:3]
                    rr = sm[:, 3:4]
                    p2 = sm[:, 4:5]
                    nc.vector.tensor_sub(d21, mx[:, 1:2], mx[:, 0:1])
                    nc.scalar.activation(e2, d21,
                                         mybir.ActivationFunctionType.Exp)
                    nc.vector.tensor_scalar_add(den, e2, 1.0)
                    nc.vector.reciprocal(rr, den)
                    nc.vector.tensor_mul(p2, e2, rr)

                    # gate row = (lg==v1)*p1 + (lg==v2)*p2  (zeros elsewhere)
                    g1 = wpool.tile([P, E], f32)
                    g2 = wpool.tile([P, E], f32)
                    nc.vector.scalar_tensor_tensor(
                        out=g1[:], in0=lg[:], scalar=mx[:, 0:1],
                        in1=rr.to_broadcast([P, E]),
                        op0=mybir.AluOpType.is_equal, op1=mybir.AluOpType.mult)
                    nc.vector.scalar_tensor_tensor(
                        out=g2[:], in0=lg[:], scalar=mx[:, 1:2],
                        in1=p2.to_broadcast([P, E]),
                        op0=mybir.AluOpType.is_equal, op1=mybir.AluOpType.mult)
                    nc.vector.tensor_add(g1[:], g1[:], g2[:])
                    nc.sync.dma_start(out=gate[t * P:(t + 1) * P, :], in_=g1[:])
    if not nc.is_finalized():
        nc.finalize()
    return nc


def kernel(x, W, b):
    global last_exec_time_ns
    nc = _CACHE.get("nc")
    if nc is None:
        nc = _build()
        _CACHE["nc"] = nc

    xf = np.ascontiguousarray(np.asarray(x, dtype=np.float32)).reshape(TOK, DM)
    Wf = np.asarray(W, dtype=np.float32)
    bf = np.asarray(b, dtype=np.float32)

    # win[p, k*E+e] = W[e, k*P+p]
    warr = np.ascontiguousarray(
        Wf.T.reshape(ND, P, E).transpose(1, 0, 2)).reshape(P, ND * E)
    barr = np.ascontiguousarray(bf.reshape(1, E))

    in_maps = []
    for c in range(NCORES):
        xc = xf[c * TPC:(c + 1) * TPC]                 # [2048, 2048]
        # xin[k, p, tok] = x[tok, k*P+p]
        xt = np.ascontiguousarray(
            xc.reshape(TPC, ND, P).transpose(1, 2, 0))
        in_maps.append({"xin": xt, "win": warr, "bin": barr})

    trace = bool(int(os.environ.get("KERNEL_TRACE", "0")))
    res = run_bass_kernel_spmd(nc, in_maps, list(range(NCORES)), trace=trace)
    last_exec_time_ns = res.exec_time_ns

    gate_full = np.concatenate(
        [res.results[c]["gate"] for c in range(NCORES)], axis=0)
    idx_full = np.concatenate(
        [res.results[c]["idx"] for c in range(NCORES)], axis=0)
    return (gate_full.reshape(B, S, E),
            idx_full.reshape(B, S, TOPK).astype(np.int32))
